# revision 13
# baseline (speedup 1.0000x reference)
"""Trainium2 Bass kernel for nn_AddWithCarryNetwork (B=2048, N=4096, H=32).

Math: the reference scans bits LSB->MSB with a tiny MLP per step:
  h = sigmoid([x_i, y_i, c] @ W1 + b1);  out = sigmoid(h @ W2 + b2)
  sum_i = out[:,0], c' = out[:,1]
Because x_i, y_i are exactly {0,1}, each step applies one of FOUR fixed
scalar maps c -> (sum, c').  Over the reachable carry interval (derived
from the weights alone) each map is affine in c to ~1e-3, so the scan
becomes the DVE's native tensor_tensor_scan linear recurrence
  c_t = BE_t*c_{t-1} + AL_t,         sum_t = SA_t + SB*c_{t-1}
The per-(row,bit) coefficients BE/AL/SA are affine in (x, y) (3-dof
least-squares over the 4 cases; the small x*y interaction is dropped)
and SB is well-approximated by a constant.  End-to-end max error vs the
exact reference is ~7e-3 (gate 2e-2).

Sharding: data-parallel over batch, 256 rows/core x 8 cores.  Everything
runs in bf16 (x, y are exact in bf16): tensor_scalar hits the DVE 4x
mode (1.2us/[128,4096]), tensor_tensor the 2x mode (2.3us).  X-affine
terms run on the Activation engine in parallel with the DVE.
"""

import numpy as np
import ml_dtypes

import concourse.bass as bass
import concourse.mybir as mybir
from concourse.bass_utils import run_bass_kernel_spmd

BF16 = ml_dtypes.bfloat16
B, N = 2048, 4096
N_CORES = 8
ROWS = B // N_CORES          # 256 rows per core
TILE_P = 128                 # SBUF partition dim
TILES = ROWS // TILE_P       # 2 tiles per core


def _sigmoid(z):
    return 1.0 / (1.0 + np.exp(-z))


def _fit_coeffs(W1, b1, W2, b2):
    """Weights-only preprocessing: affine fit of the 4 case maps.

    Returns 3-dof (c0, cx, cy) coefficients for BE (carry slope), AL
    (carry offset), SA (sum offset), plus the constant SB (sum slope).
    """
    W1 = W1.astype(np.float64); b1 = b1.astype(np.float64)
    W2 = W2.astype(np.float64); b2 = b2.astype(np.float64)
    cases = [(0, 0), (0, 1), (1, 0), (1, 1)]
    U = np.stack([xb * W1[0] + yb * W1[1] + b1 for xb, yb in cases])  # [4,H]
    v = W1[2]

    def step_all(c):
        c = np.asarray(c, np.float64)
        h = _sigmoid(U[:, None, :] + v[None, None, :] * c.reshape(1, -1, 1))
        z = h @ W2 + b2
        return _sigmoid(z[..., 1]), _sigmoid(z[..., 0])  # carry, sum

    lo, hi = 0.0, 0.0
    for _ in range(30):
        grid = np.linspace(min(lo, 0.0), max(hi, 0.0), 201)
        cg, _sg = step_all(grid)
        nlo, nhi = float(cg.min()), float(cg.max())
        if abs(nlo - lo) < 1e-9 and abs(nhi - hi) < 1e-9:
            break
        lo, hi = min(lo, nlo), max(hi, nhi)

    grid = np.unique(np.concatenate([[0.0], np.linspace(min(lo, 0.0), hi, 513)]))
    cg, sg = step_all(grid)
    A = np.stack([np.ones_like(grid), grid], 1)
    beta = np.zeros(4); alpha = np.zeros(4); sa = np.zeros(4); sb = np.zeros(4)
    for k in range(4):
        (alpha[k], beta[k]), *_ = np.linalg.lstsq(A, cg[k], rcond=None)
        (sa[k], sb[k]), *_ = np.linalg.lstsq(A, sg[k], rcond=None)

    D = np.array([[1, 0, 0], [1, 0, 1], [1, 1, 0], [1, 1, 1]], np.float64)

    def fit3(vals):
        coef, *_ = np.linalg.lstsq(D, vals, rcond=None)
        return coef

    b0, bx, by = fit3(beta)
    a0, ax, ay = fit3(alpha)
    s0, sx, sy = fit3(sa)
    return dict(b0=b0, bx=bx, by=by, a0=a0, ax=ax, ay=ay,
                s0=s0, sx=sx, sy=sy, sbbar=float(sb.mean()))


def _build_nc(co):
    """Build the SPMD Bass program (identical on all 8 cores)."""
    nc = bass.Bass()
    dt = mybir.dt.bfloat16
    op = mybir.AluOpType
    Act = mybir.ActivationFunctionType

    xb = nc.declare_dram_parameter("xb", [ROWS, N], dt, isOutput=False)
    yb = nc.declare_dram_parameter("yb", [ROWS, N], dt, isOutput=False)
    out = nc.declare_dram_parameter("out", [ROWS, N], dt, isOutput=True)

    # Register activation bias constants (same mechanism as Bass.__init__).
    for key in ("b0", "a0", "s0"):
        v = float(co[key])
        if (mybir.dt.float32, v) not in nc.const_aps.aps:
            t = nc.alloc_sbuf_tensor(f"const-bias-{key}", [128, 1],
                                     mybir.dt.float32)
            nc.gpsimd.memset(t.ap(), v)
            nc.const_aps.aps[(mybir.dt.float32, v)] = t.ap()
    nc.all_engine_barrier()

    from contextlib import ExitStack
    with ExitStack() as ctx:
        sb = lambda nm, f=N: ctx.enter_context(
            nc.sbuf_tensor(nm, [TILE_P, f], dt))
        X = [sb(f"Xt{t}") for t in range(TILES)]
        Y = [sb(f"Yt{t}") for t in range(TILES)]
        TBE = [sb(f"TBEt{t}") for t in range(TILES)]
        TAL = [sb(f"TALt{t}") for t in range(TILES)]
        TSA = [sb(f"TSAt{t}") for t in range(TILES)]
        YB = [sb(f"YBt{t}") for t in range(TILES)]
        S = [sb(f"St{t}") for t in range(TILES)]
        Ysc2, BE, AL, SAb, SM = sb("Ysc2"), sb("BE"), sb("AL"), sb("SAb"), sb("SM")
        Ysc1 = sb("Ysc1")
        Cpad = sb("Cpad", N + 2)

        dmax = [ctx.enter_context(nc.semaphore(f"dmax{t}")) for t in range(TILES)]
        dmay = [ctx.enter_context(nc.semaphore(f"dmay{t}")) for t in range(TILES)]
        dmao = [ctx.enter_context(nc.semaphore(f"dmao{t}")) for t in range(TILES)]
        acts = ctx.enter_context(nc.semaphore("acts"))
        dvp = ctx.enter_context(nc.semaphore("dvp"))

        OPS_PER_TILE = 8  # DVE ops per tile (after the one-time memset)

        with nc.Block() as block:

            @block.sync
            def _(sync):
                for t in range(TILES):
                    r = slice(t * TILE_P, (t + 1) * TILE_P)
                    sync.dma_start(X[t][:, :], xb[r, :]).then_inc(dmax[t], 16)
                for t in range(TILES):
                    r = slice(t * TILE_P, (t + 1) * TILE_P)
                    sync.wait_ge(dvp, 1 + OPS_PER_TILE * (t + 1))
                    sync.dma_start(out[r, :], S[t][:, :]).then_inc(dmao[t], 16)
                for t in range(TILES):
                    sync.wait_ge(dmao[t], 16)

            @block.scalar
            def _(scalar):
                # Y loads ride the Activation DGE ring, parallel to SP's X.
                for t in range(TILES):
                    r = slice(t * TILE_P, (t + 1) * TILE_P)
                    scalar.dma_start(Y[t][:, :], yb[r, :]).then_inc(dmay[t], 16)
                for t in range(TILES):
                    scalar.wait_ge(dmax[t], 16)
                    nc.scalar.activation(TBE[t][:, :], X[t][:, :], Act.Identity,
                                         bias=float(co["b0"]), scale=float(co["bx"])
                                         ).then_inc(acts, 1)
                    nc.scalar.activation(TAL[t][:, :], X[t][:, :], Act.Identity,
                                         bias=float(co["a0"]), scale=float(co["ax"])
                                         ).then_inc(acts, 1)
                    scalar.wait_ge(dmay[t], 16)
                    nc.scalar.activation(YB[t][:, :], Y[t][:, :], Act.Copy,
                                         bias=0.0, scale=float(co["by"])
                                         ).then_inc(acts, 1)
                    nc.scalar.activation(TSA[t][:, :], X[t][:, :], Act.Identity,
                                         bias=float(co["s0"]), scale=float(co["sx"])
                                         ).then_inc(acts, 1)

            @block.vector
            def _(vector):
                k = [0]

                def done(instr):
                    instr.then_inc(dvp, 1)
                    k[0] += 1

                def barrier():
                    vector.wait_ge(dvp, k[0])

                # Cpad[:,0] stays 0 forever: carry-in of bit 0 is 0.
                done(nc.vector.memset(Cpad[:, 0:1], 0.0))
                for t in range(TILES):
                    vector.wait_ge(dmay[t], 16)
                    barrier()
                    # Ysc2 = ay*Y  (4x-mode tensor_scalar)
                    done(nc.vector.tensor_scalar(
                        Ysc2[:, :], Y[t][:, :], float(co["ay"]), None, op.mult))
                    vector.wait_ge(acts, 4 * t + 1)
                    vector.wait_ge(acts, 4 * t + 3)  # YB[t] ready
                    done(nc.vector.tensor_tensor(
                        BE[:, :], TBE[t][:, :], YB[t][:, :], op.add))
                    vector.wait_ge(acts, 4 * t + 2)
                    barrier()
                    done(nc.vector.tensor_tensor(
                        AL[:, :], TAL[t][:, :], Ysc2[:, :], op.add))
                    barrier()
                    # carries (inclusive), shifted by one: Cpad[i+1] = c_{i+1}
                    done(nc.vector.tensor_tensor_scan(
                        Cpad[:, 1:N + 1], BE[:, :], AL[:, :], 0.0,
                        op.mult, op.add))
                    # Ysc1 = sy*Y (safe: nothing reads Ysc1 before this tile)
                    done(nc.vector.tensor_scalar(
                        Ysc1[:, :], Y[t][:, :], float(co["sy"]), None, op.mult))
                    vector.wait_ge(acts, 4 * t + 4)
                    barrier()
                    done(nc.vector.tensor_tensor(
                        SAb[:, :], TSA[t][:, :], Ysc1[:, :], op.add))
                    barrier()
                    # SM = SBbar * c_exclusive  (Cpad[:,0:N] = carry-in per bit)
                    done(nc.vector.tensor_scalar(
                        SM[:, :], Cpad[:, 0:N], float(co["sbbar"]), None,
                        op.mult))
                    barrier()
                    done(nc.vector.tensor_tensor(
                        S[t][:, :], SM[:, :], SAb[:, :], op.add))
                assert k[0] == 1 + OPS_PER_TILE * TILES

    return nc


def _run(x, y, W1, b1, W2, b2, **spmd_kwargs):
    co = _fit_coeffs(W1, b1, W2, b2)

    # LSB-first bit order, bf16 (0/1 are exact), shard batch across 8 cores.
    xf = np.ascontiguousarray(x[:, ::-1]).astype(BF16)
    yf = np.ascontiguousarray(y[:, ::-1]).astype(BF16)

    nc = _build_nc(co)
    in_maps = [
        {"xb": xf[i * ROWS:(i + 1) * ROWS], "yb": yf[i * ROWS:(i + 1) * ROWS]}
        for i in range(N_CORES)
    ]
    res = run_bass_kernel_spmd(nc, in_maps, core_ids=list(range(N_CORES)),
                               **spmd_kwargs)
    outs = [res.results[i]["out"] for i in range(N_CORES)]
    full = np.concatenate(outs, axis=0).astype(np.float32)
    return np.ascontiguousarray(full[:, ::-1]), res


def kernel(x, y, W1, b1, W2, b2):
    return _run(x, y, W1, b1, W2, b2)[0]


# revision 16
# speedup vs baseline: 1.2731x; 1.2731x over previous
"""Trainium2 Bass kernel for nn_AddWithCarryNetwork (B=2048, N=4096, H=32).

Math: the reference scans bits LSB->MSB with a tiny MLP per step:
  h = sigmoid([x_i, y_i, c] @ W1 + b1);  out = sigmoid(h @ W2 + b2)
  sum_i = out[:,0], c' = out[:,1]
Because x_i, y_i are exactly {0,1}, each step applies one of FOUR fixed
scalar maps c -> (sum, c').  Over the reachable carry interval (derived
from the weights alone) each map is affine in c to ~1e-3, so the scan
becomes the DVE's native tensor_tensor_scan linear recurrence
  c_t = BE_t*c_{t-1} + AL_t,         sum_t = SA_t + SB*c_{t-1}
The per-(row,bit) coefficients BE/AL/SA are affine in (x, y) (3-dof
least-squares over the 4 cases; the small x*y interaction is dropped)
and SB is well-approximated by a constant.  End-to-end max error vs the
exact reference is ~7e-3 (gate 2e-2).

Sharding: data-parallel over batch, 256 rows/core x 8 cores.  Everything
runs in bf16 (x, y are exact in bf16): tensor_scalar hits the DVE 4x
mode (1.2us/[128,4096]), tensor_tensor the 2x mode (2.3us).  X-affine
terms run on the Activation engine in parallel with the DVE.
"""

import numpy as np
import ml_dtypes

import concourse.bass as bass
import concourse.mybir as mybir
from concourse.bass_utils import run_bass_kernel_spmd

BF16 = ml_dtypes.bfloat16
B, N = 2048, 4096
N_CORES = 8
ROWS = B // N_CORES          # 256 rows per core
TILE_P = 128                 # SBUF partition dim
TILES = ROWS // TILE_P       # 2 tiles per core


def _sigmoid(z):
    return 1.0 / (1.0 + np.exp(-z))


def _fit_coeffs(W1, b1, W2, b2):
    """Weights-only preprocessing: affine fit of the 4 case maps.

    Returns 3-dof (c0, cx, cy) coefficients for BE (carry slope), AL
    (carry offset), SA (sum offset), plus the constant SB (sum slope).
    """
    W1 = W1.astype(np.float64); b1 = b1.astype(np.float64)
    W2 = W2.astype(np.float64); b2 = b2.astype(np.float64)
    cases = [(0, 0), (0, 1), (1, 0), (1, 1)]
    U = np.stack([xb * W1[0] + yb * W1[1] + b1 for xb, yb in cases])  # [4,H]
    v = W1[2]

    def step_all(c):
        c = np.asarray(c, np.float64)
        h = _sigmoid(U[:, None, :] + v[None, None, :] * c.reshape(1, -1, 1))
        z = h @ W2 + b2
        return _sigmoid(z[..., 1]), _sigmoid(z[..., 0])  # carry, sum

    lo, hi = 0.0, 0.0
    for _ in range(30):
        grid = np.linspace(min(lo, 0.0), max(hi, 0.0), 201)
        cg, _sg = step_all(grid)
        nlo, nhi = float(cg.min()), float(cg.max())
        if abs(nlo - lo) < 1e-9 and abs(nhi - hi) < 1e-9:
            break
        lo, hi = min(lo, nlo), max(hi, nhi)

    grid = np.unique(np.concatenate([[0.0], np.linspace(min(lo, 0.0), hi, 513)]))
    cg, sg = step_all(grid)
    A = np.stack([np.ones_like(grid), grid], 1)
    beta = np.zeros(4); alpha = np.zeros(4); sa = np.zeros(4); sb = np.zeros(4)
    for k in range(4):
        (alpha[k], beta[k]), *_ = np.linalg.lstsq(A, cg[k], rcond=None)
        (sa[k], sb[k]), *_ = np.linalg.lstsq(A, sg[k], rcond=None)

    D = np.array([[1, 0, 0], [1, 0, 1], [1, 1, 0], [1, 1, 1]], np.float64)

    def fit3(vals):
        coef, *_ = np.linalg.lstsq(D, vals, rcond=None)
        return coef

    b0, bx, by = fit3(beta)
    a0, ax, ay = fit3(alpha)
    s0, sx, sy = fit3(sa)
    return dict(b0=b0, bx=bx, by=by, a0=a0, ax=ax, ay=ay,
                s0=s0, sx=sx, sy=sy, sbbar=float(sb.mean()))


def _build_nc(co):
    """Build the SPMD Bass program (identical on all 8 cores)."""
    nc = bass.Bass()
    dt = mybir.dt.bfloat16
    op = mybir.AluOpType
    Act = mybir.ActivationFunctionType

    xb = nc.declare_dram_parameter("xb", [ROWS, N], dt, isOutput=False)
    yb = nc.declare_dram_parameter("yb", [ROWS, N], dt, isOutput=False)
    out = nc.declare_dram_parameter("out", [ROWS, N], dt, isOutput=True)

    # Register activation bias constants (same mechanism as Bass.__init__).
    for key in ("b0", "a0", "s0"):
        v = float(co[key])
        if (mybir.dt.float32, v) not in nc.const_aps.aps:
            t = nc.alloc_sbuf_tensor(f"const-bias-{key}", [128, 1],
                                     mybir.dt.float32)
            nc.gpsimd.memset(t.ap(), v)
            nc.const_aps.aps[(mybir.dt.float32, v)] = t.ap()
    nc.all_engine_barrier()

    from contextlib import ExitStack
    with ExitStack() as ctx:
        sb = lambda nm, f=N: ctx.enter_context(
            nc.sbuf_tensor(nm, [TILE_P, f], dt))
        X = [sb(f"Xt{t}") for t in range(TILES)]
        Y = [sb(f"Yt{t}") for t in range(TILES)]
        TBE = [sb(f"TBEt{t}") for t in range(TILES)]
        TAL = [sb(f"TALt{t}") for t in range(TILES)]
        TSA = [sb(f"TSAt{t}") for t in range(TILES)]
        YB = [sb(f"YBt{t}") for t in range(TILES)]
        S = [sb(f"St{t}") for t in range(TILES)]
        Ysc2, BE, AL, SAb, SM = sb("Ysc2"), sb("BE"), sb("AL"), sb("SAb"), sb("SM")
        Ysc1 = sb("Ysc1")
        Cpad = sb("Cpad", N + 2)

        dmax = [ctx.enter_context(nc.semaphore(f"dmax{t}")) for t in range(TILES)]
        dmay = [ctx.enter_context(nc.semaphore(f"dmay{t}")) for t in range(TILES)]
        dmao = [ctx.enter_context(nc.semaphore(f"dmao{t}")) for t in range(TILES)]
        acts = ctx.enter_context(nc.semaphore("acts"))
        dvp = ctx.enter_context(nc.semaphore("dvp"))

        OPS_PER_TILE = 9  # DVE ops per tile (after the one-time memset)

        with nc.Block() as block:

            @block.sync
            def _(sync):
                for t in range(TILES):
                    r = slice(t * TILE_P, (t + 1) * TILE_P)
                    sync.dma_start(X[t][:, :], xb[r, :]).then_inc(dmax[t], 16)
                    sync.dma_start(Y[t][:, :], yb[r, :]).then_inc(dmay[t], 16)
                for t in range(TILES):
                    r = slice(t * TILE_P, (t + 1) * TILE_P)
                    sync.wait_ge(dvp, 1 + OPS_PER_TILE * (t + 1))
                    sync.dma_start(out[r, :], S[t][:, :]).then_inc(dmao[t], 16)
                for t in range(TILES):
                    sync.wait_ge(dmao[t], 16)

            @block.scalar
            def _(scalar):
                for t in range(TILES):
                    scalar.wait_ge(dmax[t], 16)
                    nc.scalar.activation(TBE[t][:, :], X[t][:, :], Act.Identity,
                                         bias=float(co["b0"]), scale=float(co["bx"])
                                         ).then_inc(acts, 1)
                    nc.scalar.activation(TAL[t][:, :], X[t][:, :], Act.Identity,
                                         bias=float(co["a0"]), scale=float(co["ax"])
                                         ).then_inc(acts, 1)
                    nc.scalar.activation(TSA[t][:, :], X[t][:, :], Act.Identity,
                                         bias=float(co["s0"]), scale=float(co["sx"])
                                         ).then_inc(acts, 1)

            @block.vector
            def _(vector):
                k = [0]

                def done(instr):
                    instr.then_inc(dvp, 1)
                    k[0] += 1

                def barrier():
                    vector.wait_ge(dvp, k[0])

                # Cpad[:,0] stays 0 forever: carry-in of bit 0 is 0.
                done(nc.vector.memset(Cpad[:, 0:1], 0.0))
                for t in range(TILES):
                    vector.wait_ge(dmay[t], 16)
                    barrier()
                    # 4x-mode tensor_scalar Y-terms (no ACT dependency)
                    done(nc.vector.tensor_scalar(
                        YB[t][:, :], Y[t][:, :], float(co["by"]), None, op.mult))
                    done(nc.vector.tensor_scalar(
                        Ysc2[:, :], Y[t][:, :], float(co["ay"]), None, op.mult))
                    done(nc.vector.tensor_scalar(
                        Ysc1[:, :], Y[t][:, :], float(co["sy"]), None, op.mult))
                    vector.wait_ge(acts, 3 * t + 1)
                    barrier()
                    done(nc.vector.tensor_tensor(
                        BE[:, :], TBE[t][:, :], YB[t][:, :], op.add))
                    vector.wait_ge(acts, 3 * t + 2)
                    done(nc.vector.tensor_tensor(
                        AL[:, :], TAL[t][:, :], Ysc2[:, :], op.add))
                    barrier()
                    # carries (inclusive), shifted by one: Cpad[i+1] = c_{i+1}
                    done(nc.vector.tensor_tensor_scan(
                        Cpad[:, 1:N + 1], BE[:, :], AL[:, :], 0.0,
                        op.mult, op.add))
                    vector.wait_ge(acts, 3 * t + 3)
                    done(nc.vector.tensor_tensor(
                        SAb[:, :], TSA[t][:, :], Ysc1[:, :], op.add))
                    barrier()
                    # SM = SBbar * c_exclusive  (Cpad[:,0:N] = carry-in per bit)
                    done(nc.vector.tensor_scalar(
                        SM[:, :], Cpad[:, 0:N], float(co["sbbar"]), None,
                        op.mult))
                    barrier()
                    done(nc.vector.tensor_tensor(
                        S[t][:, :], SM[:, :], SAb[:, :], op.add))
                assert k[0] == 1 + OPS_PER_TILE * TILES

    return nc


def _run(x, y, W1, b1, W2, b2, **spmd_kwargs):
    co = _fit_coeffs(W1, b1, W2, b2)

    # LSB-first bit order, bf16 (0/1 are exact), shard batch across 8 cores.
    xf = np.ascontiguousarray(x[:, ::-1]).astype(BF16)
    yf = np.ascontiguousarray(y[:, ::-1]).astype(BF16)

    nc = _build_nc(co)
    in_maps = [
        {"xb": xf[i * ROWS:(i + 1) * ROWS], "yb": yf[i * ROWS:(i + 1) * ROWS]}
        for i in range(N_CORES)
    ]
    res = run_bass_kernel_spmd(nc, in_maps, core_ids=list(range(N_CORES)),
                               **spmd_kwargs)
    outs = [res.results[i]["out"] for i in range(N_CORES)]
    full = np.concatenate(outs, axis=0).astype(np.float32)
    return np.ascontiguousarray(full[:, ::-1]), res


def kernel(x, y, W1, b1, W2, b2):
    return _run(x, y, W1, b1, W2, b2)[0]


# revision 19
# speedup vs baseline: 1.3488x; 1.0595x over previous
"""Trainium2 Bass kernel for nn_AddWithCarryNetwork (B=2048, N=4096, H=32).

Math: the reference scans bits LSB->MSB with a tiny MLP per step:
  h = sigmoid([x_i, y_i, c] @ W1 + b1);  out = sigmoid(h @ W2 + b2)
  sum_i = out[:,0], c' = out[:,1]
Because x_i, y_i are exactly {0,1}, each step applies one of FOUR fixed
scalar maps c -> (sum, c').  Over the reachable carry interval (derived
from the weights alone) each map is affine in c to ~1e-3, so the scan
becomes the DVE's native tensor_tensor_scan linear recurrence
  c_t = BE_t*c_{t-1} + AL_t,         sum_t = (SBbar*c_{t-1} + s0) + SA_t
The per-(row,bit) coefficients BE/AL/SA are affine in (x, y) (3-dof
least-squares over the 4 cases; the small x*y interaction is dropped)
and SB is well-approximated by a constant.  End-to-end max error vs the
exact reference is ~7e-3 (gate 2e-2).

Engine split per [128, 4096] bf16 tile:
  ACT   x-terms + one y-term via Copy-with-scale (3.6us each)
  DVE   y-terms via 4x-mode tensor_scalar (1.2us), adds via 2x-mode
        tensor_tensor (2.3us), tensor_tensor_scan (8.7us)
  PE    the BE = bx*X + (by*Y+b0) add as identity-matmul accumulation
        into PSUM; the scan reads BE straight from PSUM (fp32)
Sharding: data-parallel over batch, 256 rows/core x 8 cores.
"""

import numpy as np
import ml_dtypes

import concourse.bass as bass
import concourse.mybir as mybir
from concourse.bass_utils import run_bass_kernel_spmd

BF16 = ml_dtypes.bfloat16
B, N = 2048, 4096
N_CORES = 8
ROWS = B // N_CORES          # 256 rows per core
TILE_P = 128                 # SBUF partition dim
TILES = ROWS // TILE_P       # 2 tiles per core
H = N // 2                   # half-tile split for the pipeline head
CHUNK = 512                  # PSUM bank: 512 fp32 per partition
NCHUNK = N // CHUNK


def _sigmoid(z):
    return 1.0 / (1.0 + np.exp(-z))


def _fit_coeffs(W1, b1, W2, b2):
    """Weights-only preprocessing: affine fit of the 4 case maps."""
    W1 = W1.astype(np.float64); b1 = b1.astype(np.float64)
    W2 = W2.astype(np.float64); b2 = b2.astype(np.float64)
    cases = [(0, 0), (0, 1), (1, 0), (1, 1)]
    U = np.stack([xb * W1[0] + yb * W1[1] + b1 for xb, yb in cases])  # [4,H]
    v = W1[2]

    def step_all(c):
        c = np.asarray(c, np.float64)
        h = _sigmoid(U[:, None, :] + v[None, None, :] * c.reshape(1, -1, 1))
        z = h @ W2 + b2
        return _sigmoid(z[..., 1]), _sigmoid(z[..., 0])  # carry, sum

    lo, hi = 0.0, 0.0
    for _ in range(30):
        grid = np.linspace(min(lo, 0.0), max(hi, 0.0), 201)
        cg, _sg = step_all(grid)
        nlo, nhi = float(cg.min()), float(cg.max())
        if abs(nlo - lo) < 1e-9 and abs(nhi - hi) < 1e-9:
            break
        lo, hi = min(lo, nlo), max(hi, nhi)

    grid = np.unique(np.concatenate([[0.0], np.linspace(min(lo, 0.0), hi, 513)]))
    cg, sg = step_all(grid)
    A = np.stack([np.ones_like(grid), grid], 1)
    beta = np.zeros(4); alpha = np.zeros(4); sa = np.zeros(4); sb = np.zeros(4)
    for k in range(4):
        (alpha[k], beta[k]), *_ = np.linalg.lstsq(A, cg[k], rcond=None)
        (sa[k], sb[k]), *_ = np.linalg.lstsq(A, sg[k], rcond=None)

    D = np.array([[1, 0, 0], [1, 0, 1], [1, 1, 0], [1, 1, 1]], np.float64)

    def fit3(vals):
        coef, *_ = np.linalg.lstsq(D, vals, rcond=None)
        return coef

    b0, bx, by = fit3(beta)
    a0, ax, ay = fit3(alpha)
    s0, sx, sy = fit3(sa)
    return dict(b0=b0, bx=bx, by=by, a0=a0, ax=ax, ay=ay,
                s0=s0, sx=sx, sy=sy, sbbar=float(sb.mean()))


def _build_nc(co):
    """Build the SPMD Bass program (identical on all 8 cores)."""
    nc = bass.Bass()
    dt = mybir.dt.bfloat16
    f32 = mybir.dt.float32
    op = mybir.AluOpType
    Act = mybir.ActivationFunctionType

    xb = nc.declare_dram_parameter("xb", [ROWS, N], dt, isOutput=False)
    yb = nc.declare_dram_parameter("yb", [ROWS, N], dt, isOutput=False)
    ident = nc.declare_dram_parameter("ident", [TILE_P, TILE_P], dt,
                                      isOutput=False)
    out = nc.declare_dram_parameter("out", [ROWS, N], dt, isOutput=True)

    from contextlib import ExitStack
    with ExitStack() as ctx:
        sb = lambda nm, f=N: ctx.enter_context(
            nc.sbuf_tensor(nm, [TILE_P, f], dt))
        X = [sb(f"Xt{t}") for t in range(TILES)]
        Y = [sb(f"Yt{t}") for t in range(TILES)]
        TAL = [sb(f"TALt{t}") for t in range(TILES)]
        TSA = [sb(f"TSAt{t}") for t in range(TILES)]
        YS1 = [sb(f"YS1t{t}") for t in range(TILES)]
        TBE = sb("TBE")
        S = [sb(f"St{t}") for t in range(TILES)]
        YBE, YAL, AL, SAb, SM = sb("YBE"), sb("YAL"), sb("AL"), sb("SAb"), sb("SM")
        Cpad = sb("Cpad", N + 2)
        Id = ctx.enter_context(nc.sbuf_tensor("Id", [TILE_P, TILE_P], dt))
        PS = ctx.enter_context(nc.psum_tensor("PS", [TILE_P, N], f32))

        sem = lambda nm: ctx.enter_context(nc.semaphore(nm))
        dmid = sem("dmid")
        dmax0a, dmax0b, dmay0a, dmay0b = (sem("dmax0a"), sem("dmax0b"),
                                          sem("dmay0a"), sem("dmay0b"))
        dmax1, dmay1 = sem("dmax1"), sem("dmay1")
        dmao = [sem(f"dmao{t}") for t in range(TILES)]
        acts = sem("acts")
        dvp = sem("dvp")
        pes = sem("pes")

        # dvp indices (1-based, see DVE stream below)
        DV_V4 = [6, 13]    # scan done, per tile
        DV_OUT = [11, 16]  # S ready, per tile
        DV_YBE = [3, 8]    # YBE ready, per tile
        # acts indices: t0: TBEa=1, TBEb=2, TAL=3, TSA=4, YS1=5
        #               t1: TBE=6, TAL=7, TSA=8, YS1=9
        ACT_TBE = [2, 6]
        ACT_TAL = [3, 7]
        ACT_YS1 = [5, 9]

        with nc.Block() as block:

            @block.sync
            def _(sync):
                sync.dma_start(Id[:, :], ident[:, :]).then_inc(dmid, 16)
                r0 = slice(0, TILE_P)
                r1 = slice(TILE_P, 2 * TILE_P)
                sync.dma_start(X[0][:, 0:H], xb[r0, 0:H]).then_inc(dmax0a, 16)
                sync.dma_start(Y[0][:, 0:H], yb[r0, 0:H]).then_inc(dmay0a, 16)
                sync.dma_start(X[0][:, H:N], xb[r0, H:N]).then_inc(dmax0b, 16)
                sync.dma_start(Y[0][:, H:N], yb[r0, H:N]).then_inc(dmay0b, 16)
                sync.dma_start(X[1][:, :], xb[r1, :]).then_inc(dmax1, 16)
                sync.dma_start(Y[1][:, :], yb[r1, :]).then_inc(dmay1, 16)
                for t, r in ((0, r0), (1, r1)):
                    sync.wait_ge(dvp, DV_OUT[t])
                    sync.dma_start(out[r, :], S[t][:, :]).then_inc(dmao[t], 16)
                for t in range(TILES):
                    sync.wait_ge(dmao[t], 16)

            @block.scalar
            def _(scalar):
                # tile 0 (first op in halves so it starts on the half-DMA)
                scalar.wait_ge(dmax0a, 16)
                nc.scalar.activation(TBE[:, 0:H], X[0][:, 0:H], Act.Copy,
                                     bias=0.0, scale=float(co["bx"])
                                     ).then_inc(acts, 1)
                scalar.wait_ge(dmax0b, 16)
                nc.scalar.activation(TBE[:, H:N], X[0][:, H:N], Act.Copy,
                                     bias=0.0, scale=float(co["bx"])
                                     ).then_inc(acts, 1)
                nc.scalar.activation(TAL[0][:, :], X[0][:, :], Act.Copy,
                                     bias=0.0, scale=float(co["ax"])
                                     ).then_inc(acts, 1)
                nc.scalar.activation(TSA[0][:, :], X[0][:, :], Act.Copy,
                                     bias=0.0, scale=float(co["sx"])
                                     ).then_inc(acts, 1)
                scalar.wait_ge(dmay0b, 16)
                nc.scalar.activation(YS1[0][:, :], Y[0][:, :], Act.Copy,
                                     bias=0.0, scale=float(co["sy"])
                                     ).then_inc(acts, 1)
                # tile 1
                scalar.wait_ge(dmax1, 16)
                # TBE is single-buffered: reuse once PE consumed it (tile0).
                scalar.wait_ge(pes, 1)
                nc.scalar.activation(TBE[:, :], X[1][:, :], Act.Copy,
                                     bias=0.0, scale=float(co["bx"])
                                     ).then_inc(acts, 1)
                nc.scalar.activation(TAL[1][:, :], X[1][:, :], Act.Copy,
                                     bias=0.0, scale=float(co["ax"])
                                     ).then_inc(acts, 1)
                nc.scalar.activation(TSA[1][:, :], X[1][:, :], Act.Copy,
                                     bias=0.0, scale=float(co["sx"])
                                     ).then_inc(acts, 1)
                scalar.wait_ge(dmay1, 16)
                nc.scalar.activation(YS1[1][:, :], Y[1][:, :], Act.Copy,
                                     bias=0.0, scale=float(co["sy"])
                                     ).then_inc(acts, 1)

            @block.tensor
            def _(tensor):
                tensor.wait_ge(dmid, 16)
                for t in range(TILES):
                    # BE(t) = I @ TBE + I @ YBE  (chunked into PSUM banks)
                    tensor.wait_ge(acts, ACT_TBE[t])
                    tensor.wait_ge(dvp, DV_YBE[t])
                    for c in range(NCHUNK):
                        cs = slice(c * CHUNK, (c + 1) * CHUNK)
                        nc.tensor.matmul(PS[:, cs], Id[:, :],
                                         TBE[:, cs], start=True, stop=False)
                        mm = nc.tensor.matmul(PS[:, cs], Id[:, :],
                                              YBE[:, cs], start=False,
                                              stop=True)
                        if c == NCHUNK - 1:
                            mm.then_inc(pes, 1)

            @block.vector
            def _(vector):
                k = [0]

                def done(instr):
                    instr.then_inc(dvp, 1)
                    k[0] += 1

                def barrier():
                    vector.wait_ge(dvp, k[0])

                byf, b0f = float(co["by"]), float(co["b0"])
                ayf, a0f = float(co["ay"]), float(co["a0"])
                sbf, s0f = float(co["sbbar"]), float(co["s0"])

                # 1: memset
                done(nc.vector.memset(Cpad[:, 0:1], 0.0))
                # 2-3: V1(t0) = YBE = by*Y + b0, in halves for the head
                vector.wait_ge(dmay0a, 16)
                done(nc.vector.tensor_scalar(YBE[:, 0:H], Y[0][:, 0:H],
                                             byf, b0f, op.mult, op.add))
                vector.wait_ge(dmay0b, 16)
                done(nc.vector.tensor_scalar(YBE[:, H:N], Y[0][:, H:N],
                                             byf, b0f, op.mult, op.add))
                # 3: V2(t0) = YAL = ay*Y + a0
                done(nc.vector.tensor_scalar(YAL[:, :], Y[0][:, :],
                                             ayf, a0f, op.mult, op.add))
                # 5: V3(t0) = AL = TAL + YAL
                vector.wait_ge(acts, ACT_TAL[0])
                barrier()
                done(nc.vector.tensor_tensor(AL[:, :], TAL[0][:, :],
                                             YAL[:, :], op.add))
                # 5: V4(t0) = scan (BE from PSUM fp32)
                vector.wait_ge(pes, 1)
                barrier()
                done(nc.vector.tensor_tensor_scan(
                    Cpad[:, 1:N + 1], PS[:, :], AL[:, :], 0.0,
                    op.mult, op.add))
                # 6: V5(t0) = SM = sbbar*c_excl + s0
                barrier()
                done(nc.vector.tensor_scalar(SM[:, :], Cpad[:, 0:N],
                                             sbf, s0f, op.mult, op.add))
                # 7: V1(t1) = YBE
                vector.wait_ge(dmay1, 16)
                done(nc.vector.tensor_scalar(YBE[:, :], Y[1][:, :],
                                             byf, b0f, op.mult, op.add))
                # 8: V2(t1) = YAL
                done(nc.vector.tensor_scalar(YAL[:, :], Y[1][:, :],
                                             ayf, a0f, op.mult, op.add))
                # 10: V6(t0) = SAb = TSA + YS1
                vector.wait_ge(acts, ACT_YS1[0])
                done(nc.vector.tensor_tensor(SAb[:, :], TSA[0][:, :],
                                             YS1[0][:, :], op.add))
                # 10: V7(t0) = S = SM + SAb
                barrier()
                done(nc.vector.tensor_tensor(S[0][:, :], SM[:, :],
                                             SAb[:, :], op.add))
                # 12: V3(t1) = AL
                vector.wait_ge(acts, ACT_TAL[1])
                barrier()
                done(nc.vector.tensor_tensor(AL[:, :], TAL[1][:, :],
                                             YAL[:, :], op.add))
                # 12: V4(t1) = scan
                vector.wait_ge(pes, 2)
                barrier()
                done(nc.vector.tensor_tensor_scan(
                    Cpad[:, 1:N + 1], PS[:, :], AL[:, :], 0.0,
                    op.mult, op.add))
                # 13: V5(t1) = SM
                barrier()
                done(nc.vector.tensor_scalar(SM[:, :], Cpad[:, 0:N],
                                             sbf, s0f, op.mult, op.add))
                # 15: V6(t1) = SAb
                vector.wait_ge(acts, ACT_YS1[1])
                done(nc.vector.tensor_tensor(SAb[:, :], TSA[1][:, :],
                                             YS1[1][:, :], op.add))
                # 15: V7(t1) = S
                barrier()
                done(nc.vector.tensor_tensor(S[1][:, :], SM[:, :],
                                             SAb[:, :], op.add))
                assert k[0] == 16, k[0]

    return nc


def _run(x, y, W1, b1, W2, b2, **spmd_kwargs):
    co = _fit_coeffs(W1, b1, W2, b2)

    # LSB-first bit order, bf16 (0/1 are exact), shard batch across 8 cores.
    xf = np.ascontiguousarray(x[:, ::-1]).astype(BF16)
    yf = np.ascontiguousarray(y[:, ::-1]).astype(BF16)
    ident = np.eye(TILE_P, dtype=BF16)

    nc = _build_nc(co)
    in_maps = [
        {"xb": xf[i * ROWS:(i + 1) * ROWS], "yb": yf[i * ROWS:(i + 1) * ROWS],
         "ident": ident}
        for i in range(N_CORES)
    ]
    res = run_bass_kernel_spmd(nc, in_maps, core_ids=list(range(N_CORES)),
                               **spmd_kwargs)
    outs = [res.results[i]["out"] for i in range(N_CORES)]
    full = np.concatenate(outs, axis=0).astype(np.float32)
    return np.ascontiguousarray(full[:, ::-1]), res


def kernel(x, y, W1, b1, W2, b2):
    return _run(x, y, W1, b1, W2, b2)[0]
